# revision 11
# baseline (speedup 1.0000x reference)
"""Causal multi-head attention on 8 Trainium2 NeuronCores.

Problem: B=4, T=2048, C=1024, H=16 heads, D=64, fp32.
Sharding: 4-way data parallel on batch x 2-way tensor parallel on heads.
Core c -> batch c//2, heads (c%2)*8 .. (c%2)*8+7.

Per-core dataflow (bf16 matmul inputs, fp32 PSUM accumulation):
  QT(d,t) = wqT.T @ xT          (d on partitions, 2 heads per 128-row tile)
  KT(d,t) likewise; V(t,d) with an appended ones column.
  ST(k,q) = KT_h.T @ QT_h per 128-key tile (K=64 contraction; the two
            heads of a pair land in one 2-bank PSUM tile)
  PT = exp(ST/8) on ScalarE, one double-width activation per key tile
       (scores are ~N(0,1): no max-subtraction needed)
  causal mask on diagonal tiles only: DVE multiply with one precomputed
       relative mask tile; diagonal tiles are narrowed to the unmasked
       query range
  AV flipped for full PE rate: per 128-query subtile qt,
       oq[qt](q, hd) += pt[:, h, qt].T @ [V_h | 1]   (out moving dim is
       only 65, contraction is the full 128 keys -> half the PE cycles
       of the [d, q]-layout formulation; column 64 collects l = sum p)
  norm on DVE: Aq(q, hd) = oq * (1/l) with l per-partition (no PE
       broadcast matmul needed in this layout)
  PE transpose Aq -> AT(d, q) per 128x128 tile (identity trick)
  yT(o,t) = woT.T @ AT   -> bf16 partial output, host sums the 2 TP cores.

The emission order software-pipelines the score->exp->out chain and streams
projection matmul "fillers" into the attention k-loop so the PE stays busy
through the ACT-bound stretches.
"""

import numpy as np
import ml_dtypes

B, T, C = 4, 2048, 1024
H, D = 16, 64
HL = 8           # local heads per core
DL = HL * D      # 512 local channels
N_CORES = 8
QB = 512         # query block (matmul moving dim)
NQB = T // QB    # 4 query blocks
NCT = C // 128   # 8 contraction tiles over C
NJ = HL // 2     # 4 head pairs
BF16 = ml_dtypes.bfloat16

_CACHE: dict = {}


def _build_nc(repeat=1):
    import contextlib

    import concourse.bass as bass
    from concourse import bacc, mybir, tile

    f32 = mybir.dt.float32
    bf16 = mybir.dt.bfloat16
    EXP = mybir.ActivationFunctionType.Exp

    nc = bacc.Bacc("TRN2", target_bir_lowering=False, debug=False)

    xT_d = nc.dram_tensor("xt", [C, T], bf16, kind="ExternalInput").ap()
    wq_d = nc.dram_tensor("wqt", [C, DL], bf16, kind="ExternalInput").ap()
    wk_d = nc.dram_tensor("wkt", [C, DL], bf16, kind="ExternalInput").ap()
    wv_d = nc.dram_tensor("wvt", [C, DL], bf16, kind="ExternalInput").ap()
    wo_d = nc.dram_tensor("wot", [DL, C], bf16, kind="ExternalInput").ap()
    yT_d = nc.dram_tensor("yt", [C, T], bf16, kind="ExternalOutput").ap()

    with tile.TileContext(nc) as tc:
        with (
            tc.tile_pool(name="const", bufs=1) as const,
            tc.tile_pool(name="ps", bufs=2, space="PSUM") as ps_pool,
            tc.tile_pool(name="oq", bufs=1, space="PSUM") as oq_pool,
            tc.tile_pool(name="aux", bufs=1, space="PSUM") as aux_pool,
            tc.tile_pool(name="pt", bufs=28) as pt_pool,
            tc.tile_pool(name="small", bufs=4) as small,
            tc.tile_pool(name="aq", bufs=4) as aq_pool,
            tc.tile_pool(name="ystage", bufs=4) as ystage,
            tc.For_i(0, repeat, 1) if repeat > 1 else contextlib.nullcontext(),
        ):
            xT_sb = const.tile([128, NCT, T], bf16)
            wq_sb = const.tile([128, NCT, DL], bf16)
            wk_sb = const.tile([128, NCT, DL], bf16)
            wv_sb = const.tile([128, NCT, DL], bf16)
            wo_sb = const.tile([128, DL // 128, C], bf16)
            QT_sb = const.tile([128, NJ, T], bf16)
            KT_sb = const.tile([128, NJ, T], bf16)
            V_sb = const.tile([128, T // 128, HL, D + 1], bf16)
            AT_sb = const.tile([128, NJ, T], bf16)
            mask_sb = const.tile([128, QB], bf16)
            ident_sb = const.tile([128, 128], bf16)

            nc.vector.memset(V_sb[:, :, :, D : D + 1], 1.0)
            # causal mask for diagonal tiles, relative layout: keep f >= p.
            # Every diagonal tile uses the same pattern on its w0: slice.
            nc.vector.memset(mask_sb[:], 1.0)
            nc.gpsimd.affine_select(
                out=mask_sb[:],
                in_=mask_sb[:],
                pattern=[[1, QB]],
                compare_op=mybir.AluOpType.is_ge,
                fill=0.0,
                base=0,
                channel_multiplier=-1,
            )
            # identity for PE transposes: keep f == p
            nc.vector.memset(ident_sb[:], 1.0)
            nc.gpsimd.affine_select(
                out=ident_sb[:],
                in_=ident_sb[:],
                pattern=[[1, 128]],
                compare_op=mybir.AluOpType.is_equal,
                fill=0.0,
                base=0,
                channel_multiplier=-1,
            )

            # input loads: the working set of attention(0,0) first (t-block 0
            # of xT, first-half K/Q weights, all of wv), spread over several
            # issuing engines so the DGE queues run in parallel
            HDL = DL // 2
            for c in range(NCT):
                cs = slice(c * 128, (c + 1) * 128)
                nc.sync.dma_start(xT_sb[:, c, 0:QB], xT_d[cs, 0:QB])
                nc.scalar.dma_start(wk_sb[:, c, 0:HDL], wk_d[cs, 0:HDL])
                nc.gpsimd.dma_start(wq_sb[:, c, 0:HDL], wq_d[cs, 0:HDL])
                nc.gpsimd.dma_start(wv_sb[:, c, :], wv_d[cs, :])
            for tb in range(1, NQB):
                ts_ = slice(tb * QB, (tb + 1) * QB)
                for c in range(NCT):
                    nc.sync.dma_start(
                        xT_sb[:, c, ts_], xT_d[c * 128 : (c + 1) * 128, ts_]
                    )
            for c in range(NCT):
                cs = slice(c * 128, (c + 1) * 128)
                nc.scalar.dma_start(wk_sb[:, c, HDL:DL], wk_d[cs, HDL:DL])
                nc.gpsimd.dma_start(wq_sb[:, c, HDL:DL], wq_d[cs, HDL:DL])
            for r in range(DL // 128):
                nc.scalar.dma_start(wo_sb[:, r, :], wo_d[r * 128 : (r + 1) * 128, :])

            def proj_qk_block(w_sb, out_sb, j, tb):
                # (dl, t) projection for head pair j, one 512-col t block
                acc = ps_pool.tile([128, QB], f32, tag="ps")
                for c in range(NCT):
                    nc.tensor.matmul(
                        acc[:],
                        lhsT=w_sb[:, c, j * 128 : (j + 1) * 128],
                        rhs=xT_sb[:, c, tb * QB : (tb + 1) * QB],
                        start=(c == 0),
                        stop=(c == NCT - 1),
                    )
                    yield
                nc.vector.tensor_copy(out_sb[:, j, tb * QB : (tb + 1) * QB], acc[:])

            def proj_v_block(tt):
                # V natural: (t, dl) for one 128-row t tile, all heads
                acc = ps_pool.tile([128, DL], f32, tag="ps")
                for c in range(NCT):
                    nc.tensor.matmul(
                        acc[:],
                        lhsT=xT_sb[:, c, tt * 128 : (tt + 1) * 128],
                        rhs=wv_sb[:, c, :],
                        start=(c == 0),
                        stop=(c == NCT - 1),
                    )
                    yield
                nc.vector.tensor_copy(
                    V_sb[:, tt, :, 0:D],
                    acc.rearrange("p (h d) -> p h d", h=HL),
                )

            def proj_y_block(qb, ob):
                q0 = qb * QB
                acc = ps_pool.tile([128, QB], f32, tag="ps")
                for r in range(DL // 128):
                    nc.tensor.matmul(
                        acc[:],
                        lhsT=wo_sb[:, r, ob * 128 : (ob + 1) * 128],
                        rhs=AT_sb[:, r, q0 : q0 + QB],
                        start=(r == 0),
                        stop=(r == DL // 128 - 1),
                    )
                    yield
                yst = ystage.tile([128, QB], bf16, tag="yst")
                nc.vector.tensor_copy(yst[:], acc[:])
                nc.sync.dma_start(
                    yT_d[ob * 128 : (ob + 1) * 128, q0 : q0 + QB], yst[:]
                )

            # ---- filler machinery: a queue of (name, generator) projection
            # blocks streamed into the attention k-loop as PE gap filler ----
            filler: dict = {"items": [], "idx": 0, "done": set()}

            def filler_add(name, gen):
                filler["items"].append((name, gen))

            def filler_pull(n):
                pulled = 0
                while pulled < n and filler["idx"] < len(filler["items"]):
                    name, gen = filler["items"][filler["idx"]]
                    try:
                        next(gen)
                        pulled += 1
                    except StopIteration:
                        filler["done"].add(name)
                        filler["idx"] += 1

            def filler_flush_until(names):
                while not all(n in filler["done"] for n in names):
                    if filler["idx"] >= len(filler["items"]):
                        missing = [n for n in names if n not in filler["done"]]
                        raise RuntimeError(f"filler queue exhausted: {missing}")
                    filler_pull(1)

            # PSUM banks allow only ONE open accumulation group at a time,
            # so the flipped-AV accumulations are emitted as *consecutive*
            # instruction groups (phase B), interleaved on the PE with the
            # NEXT block's score/exp tiles (phase A) and projection fillers.
            # A single kernel-lifetime aux bank hosts the rotating transpose
            # outputs (single-shot matmuls only, never an open group).
            aux = aux_pool.tile([128, 384], f32, tag="aux", name="aux")
            aux_state = {"n": 0}
            # transposes of finished, normalized q-subtiles are deferred so
            # the PE never waits on the DVE norm mul
            pending_tp: list = []

            def flush_tp():
                while pending_tp:
                    pending_tp.pop(0)()

            def norm_qt(j, qb, oq, s):
                # 1/l per query (l is col 64 of each accumulator slot), then
                # normalize into a bf16 staging tile; both heads at once.
                c0 = (2 * s) % 4
                rl = small.tile([128, 2, 1], f32, tag="rl", name="rl")
                rl_flat = bass.AP(
                    tensor=rl.tensor, offset=rl.offset, ap=[rl.ap[0], [1, 2]]
                )
                nc.vector.reciprocal(rl_flat, oq[:, c0 : c0 + 2, 64:65])
                aq = aq_pool.tile([128, 2, D], bf16, tag="aq", name="aq")
                rl_b = bass.AP(
                    tensor=rl.tensor,
                    offset=rl.offset,
                    ap=[rl.ap[0], rl.ap[1], [0, D]],
                )
                with nc.allow_low_precision(reason="bf16 attention out"):
                    nc.vector.tensor_mul(aq[:], oq[:, c0 : c0 + 2, 0:D], rl_b)

                def tp(aq=aq, s=s, j=j, q0=qb * QB):
                    # transpose [q, (h d)] -> [(h d), q] via matmul with the
                    # identity as moving operand; the two heads land on
                    # partition halves 0-63 / 64-127 as AT expects
                    col = 128 * (aux_state["n"] % 3)
                    aux_state["n"] += 1
                    aq_flat = bass.AP(
                        tensor=aq.tensor,
                        offset=aq.offset,
                        ap=[aq.ap[0], [1, 128]],
                    )
                    nc.tensor.matmul(
                        aux[:, col : col + 128],
                        lhsT=aq_flat,
                        rhs=ident_sb[:],
                        start=True,
                        stop=True,
                    )
                    nc.vector.tensor_copy(
                        AT_sb[:, j, q0 + s * 128 : q0 + (s + 1) * 128],
                        aux[:, col : col + 128],
                    )

                pending_tp.append(tp)

            def av_gen(j, qb):
                # phase B: flipped AV. Per query subtile s and head, one
                # consecutive accumulation group over the causal key tiles:
                #   oq[q, 0:64] += pt[:, hi, qsub].T @ [V_h | 1]
                # The moving dim is 65 while the contraction is the full 128
                # keys, so this runs at ~2x the PE rate of the [d, q]-layout
                # formulation. Column 64 collects l = sum(p).
                oq = oq_pool.tile([128, 4, D + 1], f32, tag="oq", name="oq")
                for s in range(4):
                    for hi in (0, 1):
                        slot = (2 * s + hi) % 4
                        kn = 4 * qb + s + 1
                        for k in range(kn):
                            nc.tensor.matmul(
                                oq[:, slot, :],
                                lhsT=pt_of[j, qb, k][:, hi, s * 128 : (s + 1) * 128],
                                rhs=V_sb[:, k, 2 * j + hi, :],
                                start=(k == 0),
                                stop=(k == kn - 1),
                            )
                            # the norm must be emitted before the final
                            # yield: the puller stops exactly at the yield
                            # count, so anything after the last yield would
                            # never run for non-final blocks
                            if hi == 1 and k == kn - 1:
                                norm_qt(j, qb, oq, s)
                            yield

            pt_of: dict = {}

            def pull_av(av, n):
                pulled = 0
                while pulled < n and av["gen"] is not None:
                    try:
                        next(av["gen"])
                        av["rem"] -= 1
                        pulled += 1
                    except StopIteration:
                        av["gen"] = None
                        if av["post"] is not None:
                            av["post"]()

            def attention(j, qb, prev_av, pull_n=2):
                q0 = qb * QB
                kb = (qb + 1) * (QB // 128)  # causal reach in 128-key tiles

                def emit_st(k):
                    k0 = k * 128
                    # diagonal tiles: only queries >= k0 are unmasked
                    w0 = max(0, k0 - q0)  # first valid query column
                    st = ps_pool.tile([128, 2, QB], f32, tag="st")
                    for hi, base in ((0, 0), (1, 64)):
                        nc.tensor.matmul(
                            st[:, hi, w0:QB],
                            lhsT=KT_sb[base : base + 64, j, k0 : k0 + 128],
                            rhs=QT_sb[base : base + 64, j, q0 + w0 : q0 + QB],
                            start=True,
                            stop=True,
                        )
                    pt = pt_pool.tile([128, 2, QB], bf16, tag="pt")
                    # P = exp(S / sqrt(D)); scores are O(1) so skipping the
                    # max-subtraction is safe in fp32/bf16 range.
                    nc.scalar.activation(
                        pt[:, :, w0:QB], st[:, :, w0:QB], EXP, scale=0.125
                    )
                    if k0 >= q0:
                        # tile crosses the causal diagonal: zero key > query
                        # (DVE multiply by the precomputed relative mask,
                        # broadcast over the two heads via a 0-stride dim)
                        m_ap = bass.AP(
                            tensor=mask_sb.tensor,
                            offset=mask_sb.offset,
                            ap=[mask_sb.ap[0], [0, 2], [1, QB - w0]],
                        )
                        nc.vector.tensor_mul(pt[:, :, w0:QB], pt[:, :, w0:QB], m_ap)
                    pt_of[j, qb, k] = pt

                # phase A of this block, with the previous block's phase B
                # spread evenly across the score slots
                for k in range(kb):
                    emit_st(k)
                    flush_tp()
                    if prev_av["gen"] is not None:
                        need = -(-prev_av["rem"] // (kb - k))  # ceil
                        pull_av(prev_av, need)
                    filler_pull(pull_n)
                # force-finalize: drain any remainder and take the extra
                # next() that raises StopIteration so post() always fires
                pull_av(prev_av, 1_000_000_000)

                post = None
                if j == NJ - 1:

                    def post(qb=qb):
                        # transposes must be emitted before the y fillers
                        # that read their AT output exist in the queue
                        flush_tp()
                        for ob in range(C // 128):
                            filler_add(f"y{qb}.{ob}", proj_y_block(qb, ob))

                return {"gen": av_gen(j, qb), "rem": 32 * qb + 20, "post": post}

            def run(gen):
                for _ in gen:
                    pass

            # Build the filler queue: everything except the j=0/qb=0
            # prerequisites, in rough just-in-time order.
            for qb in range(1, NQB):
                filler_add(f"kq0.{qb}k", proj_qk_block(wk_sb, KT_sb, 0, qb))
                filler_add(f"kq0.{qb}q", proj_qk_block(wq_sb, QT_sb, 0, qb))
                for tt in range(4 * qb, 4 * qb + 4):
                    filler_add(f"v{tt}", proj_v_block(tt))
            for j in range(1, NJ):
                for qb in range(NQB):
                    filler_add(f"kq{j}.{qb}k", proj_qk_block(wk_sb, KT_sb, j, qb))
                    filler_add(f"kq{j}.{qb}q", proj_qk_block(wq_sb, QT_sb, j, qb))
            # y blocks are appended only after the attention that writes
            # their AT_sb input has been emitted (program-order correctness)

            def need_attention(j, qb):
                if j == 0:
                    if qb == 0:
                        return []
                    names = [f"kq0.{t}k" for t in range(1, qb + 1)]
                    names += [f"kq0.{qb}q"]
                    names += [f"v{t}" for t in range(4, 4 * qb + 4)]
                    return names
                names = [f"kq{j}.{t}k" for t in range(qb + 1)]
                names += [f"kq{j}.{qb}q"]
                return names

            # j=0/qb=0 prerequisites emitted directly
            run(proj_qk_block(wk_sb, KT_sb, 0, 0))
            run(proj_qk_block(wq_sb, QT_sb, 0, 0))
            for tt in range(4):
                run(proj_v_block(tt))

            av = {"gen": None, "rem": 0, "post": None}
            for j in range(NJ):
                for qb in range(NQB):
                    filler_flush_until(need_attention(j, qb))
                    # hold filler reserve through (2,3) so attention(3,0)
                    # still has PE cover before its y fillers exist
                    av = attention(
                        j, qb, av, pull_n=1 if (j, qb) == (2, 3) else 2
                    )
            # drain the last block's AV, transposes, and the tail y projs
            pull_av(av, 1_000_000_000)
            flush_tp()
            filler_pull(1_000_000_000)

    nc.compile()
    return nc


def _get_nc():
    if "nc" not in _CACHE:
        _CACHE["nc"] = _build_nc()
    return _CACHE["nc"]


def _run(in_maps, trace=False):
    from concourse.bass_utils import run_bass_kernel_spmd

    nc = _get_nc()
    return run_bass_kernel_spmd(nc, in_maps, list(range(N_CORES)), trace=trace)


def _make_in_maps(x, W_Q, W_K, W_V, W_out):
    x = np.asarray(x, dtype=np.float32)
    W_Q = np.asarray(W_Q, dtype=np.float32)
    W_K = np.asarray(W_K, dtype=np.float32)
    W_V = np.asarray(W_V, dtype=np.float32)
    W_out = np.asarray(W_out, dtype=np.float32)

    in_maps = []
    for core in range(N_CORES):
        b, hh = core // 2, core % 2
        sl = slice(hh * DL, (hh + 1) * DL)
        in_maps.append(
            {
                "xt": np.ascontiguousarray(x[b].T).astype(BF16),
                "wqt": np.ascontiguousarray(W_Q[sl, :].T).astype(BF16),
                "wkt": np.ascontiguousarray(W_K[sl, :].T).astype(BF16),
                "wvt": np.ascontiguousarray(W_V[sl, :].T).astype(BF16),
                "wot": np.ascontiguousarray(W_out[:, sl].T).astype(BF16),
            }
        )
    return in_maps


def _assemble(results):
    y = np.empty((B, T, C), dtype=np.float32)
    for b in range(B):
        yT = results[2 * b]["yt"].astype(np.float32) + results[
            2 * b + 1
        ]["yt"].astype(np.float32)
        y[b] = yT.T
    return y


def kernel(x, W_Q, W_K, W_V, W_out):
    res = _run(_make_in_maps(x, W_Q, W_K, W_V, W_out), trace=False)
    return _assemble(res.results)


# revision 22
# speedup vs baseline: 1.0033x; 1.0033x over previous
"""Causal multi-head attention on 8 Trainium2 NeuronCores.

Problem: B=4, T=2048, C=1024, H=16 heads, D=64, fp32.
Sharding: 4-way data parallel on batch x 2-way tensor parallel on heads.
Core c -> batch c//2, heads (c%2)*8 .. (c%2)*8+7.

Per-core dataflow (bf16 matmul inputs, fp32 PSUM accumulation):
  QT(d,t) = wqT.T @ xT          (d on partitions, 2 heads per 128-row tile)
  KT(d,t) likewise; V(t,d) with an appended ones column.
  ST(k,q) = KT_h.T @ QT_h per 128-key tile (K=64 contraction; the two
            heads of a pair land in one 2-bank PSUM tile)
  PT = exp(ST/8) on ScalarE, one double-width activation per key tile
       (scores are ~N(0,1): no max-subtraction needed)
  causal mask on diagonal tiles only: DVE multiply with one precomputed
       relative mask tile; diagonal tiles are narrowed to the unmasked
       query range
  AV flipped for full PE rate: per 128-query subtile qt,
       oq[qt](q, hd) += pt[:, h, qt].T @ [V_h | 1]   (out moving dim is
       only 65, contraction is the full 128 keys -> half the PE cycles
       of the [d, q]-layout formulation; column 64 collects l = sum p)
  norm on DVE: Aq(q, hd) = oq * (1/l) with l per-partition (no PE
       broadcast matmul needed in this layout)
  PE transpose Aq -> AT(d, q) per 128x128 tile (identity trick)
  yT(o,t) = woT.T @ AT   -> bf16 partial output, host sums the 2 TP cores.

The emission order software-pipelines the score->exp->out chain and streams
projection matmul "fillers" into the attention k-loop so the PE stays busy
through the ACT-bound stretches.
"""

import numpy as np
import ml_dtypes

B, T, C = 4, 2048, 1024
H, D = 16, 64
HL = 8           # local heads per core
DL = HL * D      # 512 local channels
N_CORES = 8
QB = 512         # query block (matmul moving dim)
NQB = T // QB    # 4 query blocks
NCT = C // 128   # 8 contraction tiles over C
NJ = HL // 2     # 4 head pairs
BF16 = ml_dtypes.bfloat16

_CACHE: dict = {}


def _build_nc(repeat=1):
    import contextlib

    import concourse.bass as bass
    from concourse import bacc, mybir, tile

    f32 = mybir.dt.float32
    bf16 = mybir.dt.bfloat16
    EXP = mybir.ActivationFunctionType.Exp

    nc = bacc.Bacc("TRN2", target_bir_lowering=False, debug=False)

    xT_d = nc.dram_tensor("xt", [C, T], bf16, kind="ExternalInput").ap()
    wq_d = nc.dram_tensor("wqt", [C, DL], bf16, kind="ExternalInput").ap()
    wk_d = nc.dram_tensor("wkt", [C, DL], bf16, kind="ExternalInput").ap()
    wv_d = nc.dram_tensor("wvt", [C, DL], bf16, kind="ExternalInput").ap()
    wo_d = nc.dram_tensor("wot", [DL, C], bf16, kind="ExternalInput").ap()
    yT_d = nc.dram_tensor("yt", [C, T], bf16, kind="ExternalOutput").ap()

    with tile.TileContext(nc) as tc:
        with (
            tc.tile_pool(name="const", bufs=1) as const,
            tc.tile_pool(name="ps", bufs=2, space="PSUM") as ps_pool,
            tc.tile_pool(name="oq", bufs=1, space="PSUM") as oq_pool,
            tc.tile_pool(name="aux", bufs=1, space="PSUM") as aux_pool,
            tc.tile_pool(name="pt", bufs=32) as pt_pool,
            tc.tile_pool(name="small", bufs=6) as small,
            tc.tile_pool(name="aq", bufs=6) as aq_pool,
            tc.tile_pool(name="ystage", bufs=4) as ystage,
            tc.For_i(0, repeat, 1) if repeat > 1 else contextlib.nullcontext(),
        ):
            xT_sb = const.tile([128, NCT, T], bf16)
            wq_sb = const.tile([128, NCT, DL], bf16)
            wk_sb = const.tile([128, NCT, DL], bf16)
            wv_sb = const.tile([128, NCT, DL], bf16)
            wo_sb = const.tile([128, DL // 128, C], bf16)
            QT_sb = const.tile([128, NJ, T], bf16)
            KT_sb = const.tile([128, NJ, T], bf16)
            V_sb = const.tile([128, T // 128, HL, D + 1], bf16)
            AT_sb = const.tile([128, NJ, T], bf16)
            mask_sb = const.tile([128, QB], bf16)
            ident_sb = const.tile([128, 128], bf16)

            nc.vector.memset(V_sb[:, :, :, D : D + 1], 1.0)
            # causal mask for diagonal tiles, relative layout: keep f >= p.
            # Every diagonal tile uses the same pattern on its w0: slice.
            nc.vector.memset(mask_sb[:], 1.0)
            nc.gpsimd.affine_select(
                out=mask_sb[:],
                in_=mask_sb[:],
                pattern=[[1, QB]],
                compare_op=mybir.AluOpType.is_ge,
                fill=0.0,
                base=0,
                channel_multiplier=-1,
            )
            # identity for PE transposes: keep f == p
            nc.vector.memset(ident_sb[:], 1.0)
            nc.gpsimd.affine_select(
                out=ident_sb[:],
                in_=ident_sb[:],
                pattern=[[1, 128]],
                compare_op=mybir.AluOpType.is_equal,
                fill=0.0,
                base=0,
                channel_multiplier=-1,
            )

            # input loads. Each dma_start costs ~630ns of serialized HWDGE
            # descriptor-gen regardless of size, so tiles are batched into
            # multi-dim APs (partition p of chunk c reads DRAM row c*128+p):
            # one DMA per tensor region instead of one per 128-row tile.
            # The attention(0,0) working set goes first, split in half so the
            # first projection matmul can start after ~2 small transfers.
            HDL = DL // 2

            def src_ap(d_ap, row_len, n_c, col0, ncols, c0=0, row0=0):
                return bass.AP(
                    tensor=d_ap.tensor,
                    offset=(row0 + c0 * 128) * row_len + col0,
                    ap=[[row_len, 128], [row_len * 128, n_c], [1, ncols]],
                )

            def load(eng, dst, d_ap, row_len, n_c, col0, ncols, c0=0):
                eng.dma_start(dst, src_ap(d_ap, row_len, n_c, col0, ncols, c0))

            # all loads stay granular per c-tile: progressive sem arrival
            # keeps the PE p-state ramp behavior benign. x blocks 2-3 go on
            # the gpsimd (SWDGE) queue to unclog the shared HWDGE stage.
            for c in range(NCT):
                cs = slice(c * 128, (c + 1) * 128)
                nc.sync.dma_start(xT_sb[:, c, 0:QB], xT_d[cs, 0:QB])
                nc.scalar.dma_start(wk_sb[:, c, 0:HDL], wk_d[cs, 0:HDL])
                nc.gpsimd.dma_start(wq_sb[:, c, 0:HDL], wq_d[cs, 0:HDL])
                nc.gpsimd.dma_start(wv_sb[:, c, :], wv_d[cs, :])
            for tb in range(1, NQB):
                ts_ = slice(tb * QB, (tb + 1) * QB)
                for c in range(NCT):
                    nc.sync.dma_start(
                        xT_sb[:, c, ts_], xT_d[c * 128 : (c + 1) * 128, ts_]
                    )
            for c in range(NCT):
                cs = slice(c * 128, (c + 1) * 128)
                nc.scalar.dma_start(wk_sb[:, c, HDL:DL], wk_d[cs, HDL:DL])
                nc.gpsimd.dma_start(wq_sb[:, c, HDL:DL], wq_d[cs, HDL:DL])
            for r in range(DL // 128):
                nc.scalar.dma_start(wo_sb[:, r, :], wo_d[r * 128 : (r + 1) * 128, :])

            def proj_qk_block(w_sb, out_sb, j, tb):
                # (dl, t) projection for head pair j, one 512-col t block
                acc = ps_pool.tile([128, QB], f32, tag="ps")
                for c in range(NCT):
                    nc.tensor.matmul(
                        acc[:],
                        lhsT=w_sb[:, c, j * 128 : (j + 1) * 128],
                        rhs=xT_sb[:, c, tb * QB : (tb + 1) * QB],
                        start=(c == 0),
                        stop=(c == NCT - 1),
                    )
                    yield
                nc.vector.tensor_copy(out_sb[:, j, tb * QB : (tb + 1) * QB], acc[:])

            def proj_v_block(tt):
                # V natural: (t, dl) for one 128-row t tile, all heads
                acc = ps_pool.tile([128, DL], f32, tag="ps")
                for c in range(NCT):
                    nc.tensor.matmul(
                        acc[:],
                        lhsT=xT_sb[:, c, tt * 128 : (tt + 1) * 128],
                        rhs=wv_sb[:, c, :],
                        start=(c == 0),
                        stop=(c == NCT - 1),
                    )
                    yield
                nc.vector.tensor_copy(
                    V_sb[:, tt, :, 0:D],
                    acc.rearrange("p (h d) -> p h d", h=HL),
                )

            def proj_y_block(qb, ob):
                q0 = qb * QB
                acc = ps_pool.tile([128, QB], f32, tag="ps")
                for r in range(DL // 128):
                    nc.tensor.matmul(
                        acc[:],
                        lhsT=wo_sb[:, r, ob * 128 : (ob + 1) * 128],
                        rhs=AT_sb[:, r, q0 : q0 + QB],
                        start=(r == 0),
                        stop=(r == DL // 128 - 1),
                    )
                    yield
                yst = ystage.tile([128, QB], bf16, tag="yst")
                nc.vector.tensor_copy(yst[:], acc[:])
                nc.sync.dma_start(
                    yT_d[ob * 128 : (ob + 1) * 128, q0 : q0 + QB], yst[:]
                )

            # ---- filler machinery: a queue of (name, generator) projection
            # blocks streamed into the attention k-loop as PE gap filler ----
            filler: dict = {"items": [], "idx": 0, "done": set()}

            def filler_add(name, gen):
                filler["items"].append((name, gen))

            def filler_pull(n):
                pulled = 0
                while pulled < n and filler["idx"] < len(filler["items"]):
                    name, gen = filler["items"][filler["idx"]]
                    try:
                        next(gen)
                        pulled += 1
                    except StopIteration:
                        filler["done"].add(name)
                        filler["idx"] += 1

            def filler_flush_until(names):
                while not all(n in filler["done"] for n in names):
                    if filler["idx"] >= len(filler["items"]):
                        missing = [n for n in names if n not in filler["done"]]
                        raise RuntimeError(f"filler queue exhausted: {missing}")
                    filler_pull(1)

            # PSUM banks allow only ONE open accumulation group at a time,
            # so the flipped-AV accumulations are emitted as *consecutive*
            # instruction groups (phase B), interleaved on the PE with the
            # NEXT block's score/exp tiles (phase A) and projection fillers.
            # A single kernel-lifetime aux bank hosts the rotating transpose
            # outputs (single-shot matmuls only, never an open group).
            aux = aux_pool.tile([128, 384], f32, tag="aux", name="aux")
            aux_state = {"n": 0}
            # transposes of finished, normalized q-subtiles are deferred so
            # the PE never waits on the DVE norm mul
            pending_tp: list = []
            pending_tp_new: list = []

            def flush_tp(all_=False):
                while pending_tp:
                    pending_tp.pop(0)()
                pending_tp.extend(pending_tp_new)
                del pending_tp_new[:]
                if all_:
                    while pending_tp:
                        pending_tp.pop(0)()

            def norm_qt(j, qb, oq, s):
                # 1/l per query (l is col 64 of each accumulator slot), then
                # normalize into a bf16 staging tile; both heads at once.
                c0 = (2 * s) % 4
                rl = small.tile([128, 2, 1], f32, tag="rl", name="rl")
                rl_flat = bass.AP(
                    tensor=rl.tensor, offset=rl.offset, ap=[rl.ap[0], [1, 2]]
                )
                nc.vector.reciprocal(rl_flat, oq[:, c0 : c0 + 2, 64:65])
                aq = aq_pool.tile([128, 2, D], bf16, tag="aq", name="aq")
                rl_b = bass.AP(
                    tensor=rl.tensor,
                    offset=rl.offset,
                    ap=[rl.ap[0], rl.ap[1], [0, D]],
                )
                with nc.allow_low_precision(reason="bf16 attention out"):
                    nc.vector.tensor_mul(aq[:], oq[:, c0 : c0 + 2, 0:D], rl_b)

                def tp(aq=aq, s=s, j=j, q0=qb * QB):
                    # transpose [q, (h d)] -> [(h d), q] via matmul with the
                    # identity as moving operand; the two heads land on
                    # partition halves 0-63 / 64-127 as AT expects
                    col = 128 * (aux_state["n"] % 3)
                    aux_state["n"] += 1
                    aq_flat = bass.AP(
                        tensor=aq.tensor,
                        offset=aq.offset,
                        ap=[aq.ap[0], [1, 128]],
                    )
                    nc.tensor.matmul(
                        aux[:, col : col + 128],
                        lhsT=aq_flat,
                        rhs=ident_sb[:],
                        start=True,
                        stop=True,
                    )
                    nc.vector.tensor_copy(
                        AT_sb[:, j, q0 + s * 128 : q0 + (s + 1) * 128],
                        aux[:, col : col + 128],
                    )

                pending_tp_new.append(tp)

            def av_gen(j, qb):
                # phase B: flipped AV. Per query subtile s and head, one
                # consecutive accumulation group over the causal key tiles:
                #   oq[q, 0:64] += pt[:, hi, qsub].T @ [V_h | 1]
                # The moving dim is 65 while the contraction is the full 128
                # keys, so this runs at ~2x the PE rate of the [d, q]-layout
                # formulation. Column 64 collects l = sum(p).
                oq = oq_pool.tile([128, 4, D + 1], f32, tag="oq", name="oq")
                for s in range(4):
                    for hi in (0, 1):
                        slot = (2 * s + hi) % 4
                        kn = 4 * qb + s + 1
                        for k in range(kn):
                            nc.tensor.matmul(
                                oq[:, slot, :],
                                lhsT=pt_of[j, qb, k][:, hi, s * 128 : (s + 1) * 128],
                                rhs=V_sb[:, k, 2 * j + hi, :],
                                start=(k == 0),
                                stop=(k == kn - 1),
                            )
                            # the norm must be emitted before the final
                            # yield: the puller stops exactly at the yield
                            # count, so anything after the last yield would
                            # never run for non-final blocks
                            if hi == 1 and k == kn - 1:
                                norm_qt(j, qb, oq, s)
                            yield

            pt_of: dict = {}

            def pull_av(av, n):
                pulled = 0
                while pulled < n and av["gen"] is not None:
                    try:
                        next(av["gen"])
                        av["rem"] -= 1
                        pulled += 1
                    except StopIteration:
                        av["gen"] = None
                        if av["post"] is not None:
                            av["post"]()

            def attention(j, qb, prev_av, pull_n=2):
                q0 = qb * QB
                kb = (qb + 1) * (QB // 128)  # causal reach in 128-key tiles

                def emit_st(k):
                    k0 = k * 128
                    # diagonal tiles: only queries >= k0 are unmasked
                    w0 = max(0, k0 - q0)  # first valid query column
                    st = ps_pool.tile([128, 2, QB], f32, tag="st")
                    for hi, base in ((0, 0), (1, 64)):
                        nc.tensor.matmul(
                            st[:, hi, w0:QB],
                            lhsT=KT_sb[base : base + 64, j, k0 : k0 + 128],
                            rhs=QT_sb[base : base + 64, j, q0 + w0 : q0 + QB],
                            start=True,
                            stop=True,
                        )
                    pt = pt_pool.tile([128, 2, QB], bf16, tag="pt")
                    # P = exp(S / sqrt(D)); scores are O(1) so skipping the
                    # max-subtraction is safe in fp32/bf16 range.
                    nc.scalar.activation(
                        pt[:, :, w0:QB], st[:, :, w0:QB], EXP, scale=0.125
                    )
                    if k0 >= q0:
                        # tile crosses the causal diagonal: zero key > query
                        # (DVE multiply by the precomputed relative mask,
                        # broadcast over the two heads via a 0-stride dim)
                        m_ap = bass.AP(
                            tensor=mask_sb.tensor,
                            offset=mask_sb.offset,
                            ap=[mask_sb.ap[0], [0, 2], [1, QB - w0]],
                        )
                        nc.vector.tensor_mul(pt[:, :, w0:QB], pt[:, :, w0:QB], m_ap)
                    pt_of[j, qb, k] = pt

                # phase A of this block, with the previous block's phase B
                # spread evenly across the score slots
                for k in range(kb):
                    emit_st(k)
                    flush_tp()
                    if prev_av["gen"] is not None:
                        need = -(-prev_av["rem"] // (kb - k))  # ceil
                        pull_av(prev_av, need)
                    filler_pull(pull_n)
                # force-finalize: drain any remainder and take the extra
                # next() that raises StopIteration so post() always fires
                pull_av(prev_av, 1_000_000_000)

                post = None
                if j == NJ - 1:

                    def post(qb=qb):
                        # transposes must be emitted before the y fillers
                        # that read their AT output exist in the queue
                        flush_tp(all_=True)
                        for ob in range(C // 128):
                            filler_add(f"y{qb}.{ob}", proj_y_block(qb, ob))

                return {"gen": av_gen(j, qb), "rem": 32 * qb + 20, "post": post}

            def run(gen):
                for _ in gen:
                    pass

            # Build the filler queue: everything except the j=0/qb=0
            # prerequisites, in rough just-in-time order.
            for qb in range(1, NQB):
                filler_add(f"kq0.{qb}k", proj_qk_block(wk_sb, KT_sb, 0, qb))
                filler_add(f"kq0.{qb}q", proj_qk_block(wq_sb, QT_sb, 0, qb))
                for tt in range(4 * qb, 4 * qb + 4):
                    filler_add(f"v{tt}", proj_v_block(tt))
            for j in range(1, NJ):
                for qb in range(NQB):
                    filler_add(f"kq{j}.{qb}k", proj_qk_block(wk_sb, KT_sb, j, qb))
                    filler_add(f"kq{j}.{qb}q", proj_qk_block(wq_sb, QT_sb, j, qb))
            # y blocks are appended only after the attention that writes
            # their AT_sb input has been emitted (program-order correctness)

            def need_attention(j, qb):
                if j == 0:
                    if qb == 0:
                        return []
                    names = [f"kq0.{t}k" for t in range(1, qb + 1)]
                    names += [f"kq0.{qb}q"]
                    names += [f"v{t}" for t in range(4, 4 * qb + 4)]
                    return names
                names = [f"kq{j}.{t}k" for t in range(qb + 1)]
                names += [f"kq{j}.{qb}q"]
                return names

            # j=0/qb=0 prerequisites emitted directly
            run(proj_qk_block(wk_sb, KT_sb, 0, 0))
            run(proj_qk_block(wq_sb, QT_sb, 0, 0))
            for tt in range(4):
                run(proj_v_block(tt))

            av = {"gen": None, "rem": 0, "post": None}
            for j in range(NJ):
                for qb in range(NQB):
                    filler_flush_until(need_attention(j, qb))
                    # hold filler reserve through (2,3) so attention(3,0)
                    # still has PE cover before its y fillers exist
                    av = attention(
                        j, qb, av, pull_n=1 if (j, qb) == (2, 3) else 2
                    )
            # drain the last block's AV, transposes, and the tail y projs
            pull_av(av, 1_000_000_000)
            flush_tp(all_=True)
            filler_pull(1_000_000_000)

    nc.compile()
    return nc


def _get_nc():
    if "nc" not in _CACHE:
        _CACHE["nc"] = _build_nc()
    return _CACHE["nc"]


def _run(in_maps, trace=False):
    from concourse.bass_utils import run_bass_kernel_spmd

    nc = _get_nc()
    return run_bass_kernel_spmd(nc, in_maps, list(range(N_CORES)), trace=trace)


def _make_in_maps(x, W_Q, W_K, W_V, W_out):
    x = np.asarray(x, dtype=np.float32)
    W_Q = np.asarray(W_Q, dtype=np.float32)
    W_K = np.asarray(W_K, dtype=np.float32)
    W_V = np.asarray(W_V, dtype=np.float32)
    W_out = np.asarray(W_out, dtype=np.float32)

    in_maps = []
    for core in range(N_CORES):
        b, hh = core // 2, core % 2
        sl = slice(hh * DL, (hh + 1) * DL)
        in_maps.append(
            {
                "xt": np.ascontiguousarray(x[b].T).astype(BF16),
                "wqt": np.ascontiguousarray(W_Q[sl, :].T).astype(BF16),
                "wkt": np.ascontiguousarray(W_K[sl, :].T).astype(BF16),
                "wvt": np.ascontiguousarray(W_V[sl, :].T).astype(BF16),
                "wot": np.ascontiguousarray(W_out[:, sl].T).astype(BF16),
            }
        )
    return in_maps


def _assemble(results):
    y = np.empty((B, T, C), dtype=np.float32)
    for b in range(B):
        yT = results[2 * b]["yt"].astype(np.float32) + results[
            2 * b + 1
        ]["yt"].astype(np.float32)
        y[b] = yT.T
    return y


def kernel(x, W_Q, W_K, W_V, W_out):
    res = _run(_make_in_maps(x, W_Q, W_K, W_V, W_out), trace=False)
    return _assemble(res.results)


# revision 25
# speedup vs baseline: 1.0343x; 1.0309x over previous
"""Causal multi-head attention on 8 Trainium2 NeuronCores.

Problem: B=4, T=2048, C=1024, H=16 heads, D=64, fp32.
Sharding: 4-way data parallel on batch x 2-way tensor parallel on heads.
Core c -> batch c//2, heads (c%2)*8 .. (c%2)*8+7.

Per-core dataflow (bf16 matmul inputs, fp32 PSUM accumulation):
  QT(d,t) = wqT.T @ xT          (d on partitions, 2 heads per 128-row tile)
  KT(d,t) likewise; V(t,d) with an appended ones column.
  ST(k,q) = KT_h.T @ QT_h per 128-key tile (K=64 contraction; the two
            heads of a pair land in one 2-bank PSUM tile)
  PT = exp(ST/8) on ScalarE, one double-width activation per key tile
       (scores are ~N(0,1): no max-subtraction needed)
  causal mask on diagonal tiles only: DVE multiply with one precomputed
       relative mask tile; diagonal tiles are narrowed to the unmasked
       query range
  AV flipped for full PE rate: per 128-query subtile qt,
       oq[qt](q, hd) += pt[:, h, qt].T @ [V_h | 1]   (out moving dim is
       only 65, contraction is the full 128 keys -> half the PE cycles
       of the [d, q]-layout formulation; column 64 collects l = sum p)
  norm on DVE: Aq(q, hd) = oq * (1/l) with l per-partition (no PE
       broadcast matmul needed in this layout)
  PE transpose Aq -> AT(d, q) per 128x128 tile (identity trick)
  yT(o,t) = woT.T @ AT   -> bf16 partial output, host sums the 2 TP cores.

The emission order software-pipelines the score->exp->out chain and streams
projection matmul "fillers" into the attention k-loop so the PE stays busy
through the ACT-bound stretches.
"""

import numpy as np
import ml_dtypes

B, T, C = 4, 2048, 1024
H, D = 16, 64
HL = 8           # local heads per core
DL = HL * D      # 512 local channels
N_CORES = 8
QB = 512         # query block (matmul moving dim)
NQB = T // QB    # 4 query blocks
NCT = C // 128   # 8 contraction tiles over C
NJ = HL // 2     # 4 head pairs
BF16 = ml_dtypes.bfloat16

_CACHE: dict = {}


def _build_nc(repeat=1):
    import contextlib

    import concourse.bass as bass
    from concourse import bacc, mybir, tile

    f32 = mybir.dt.float32
    bf16 = mybir.dt.bfloat16
    EXP = mybir.ActivationFunctionType.Exp

    nc = bacc.Bacc("TRN2", target_bir_lowering=False, debug=False)

    xT_d = nc.dram_tensor("xt", [C, T], bf16, kind="ExternalInput").ap()
    wq_d = nc.dram_tensor("wqt", [C, DL], bf16, kind="ExternalInput").ap()
    wk_d = nc.dram_tensor("wkt", [C, DL], bf16, kind="ExternalInput").ap()
    wv_d = nc.dram_tensor("wvt", [C, DL], bf16, kind="ExternalInput").ap()
    wo_d = nc.dram_tensor("wot", [DL, C], bf16, kind="ExternalInput").ap()
    yT_d = nc.dram_tensor("yt", [C, T], bf16, kind="ExternalOutput").ap()

    with tile.TileContext(nc) as tc:
        with (
            tc.tile_pool(name="const", bufs=1) as const,
            tc.tile_pool(name="ps", bufs=2, space="PSUM") as ps_pool,
            tc.tile_pool(name="oq", bufs=1, space="PSUM") as oq_pool,
            tc.tile_pool(name="aux", bufs=1, space="PSUM") as aux_pool,
            tc.tile_pool(name="pt", bufs=32) as pt_pool,
            tc.tile_pool(name="small", bufs=6) as small,
            tc.tile_pool(name="aq", bufs=6) as aq_pool,
            tc.tile_pool(name="ystage", bufs=4) as ystage,
            tc.For_i(0, repeat, 1) if repeat > 1 else contextlib.nullcontext(),
        ):
            xT_sb = const.tile([128, NCT, T], bf16)
            wq_sb = const.tile([128, NCT, DL], bf16)
            wk_sb = const.tile([128, NCT, DL], bf16)
            wv_sb = const.tile([128, NCT, DL], bf16)
            wo_sb = const.tile([128, DL // 128, C], bf16)
            QT_sb = const.tile([128, NJ, T], bf16)
            KT_sb = const.tile([128, NJ, T], bf16)
            V_sb = const.tile([128, T // 128, HL, D + 1], bf16)
            AT_sb = const.tile([128, NJ, T], bf16)
            mask_sb = const.tile([128, QB], bf16)
            ident_sb = const.tile([128, 128], bf16)

            nc.vector.memset(V_sb[:, :, :, D : D + 1], 1.0)
            # causal mask for diagonal tiles, relative layout: keep f >= p.
            # Every diagonal tile uses the same pattern on its w0: slice.
            nc.vector.memset(mask_sb[:], 1.0)
            nc.gpsimd.affine_select(
                out=mask_sb[:],
                in_=mask_sb[:],
                pattern=[[1, QB]],
                compare_op=mybir.AluOpType.is_ge,
                fill=0.0,
                base=0,
                channel_multiplier=-1,
            )
            # identity for PE transposes: keep f == p
            nc.vector.memset(ident_sb[:], 1.0)
            nc.gpsimd.affine_select(
                out=ident_sb[:],
                in_=ident_sb[:],
                pattern=[[1, 128]],
                compare_op=mybir.AluOpType.is_equal,
                fill=0.0,
                base=0,
                channel_multiplier=-1,
            )

            # input loads. Each dma_start costs ~630ns of serialized HWDGE
            # descriptor-gen regardless of size, so tiles are batched into
            # multi-dim APs (partition p of chunk c reads DRAM row c*128+p):
            # one DMA per tensor region instead of one per 128-row tile.
            # The attention(0,0) working set goes first, split in half so the
            # first projection matmul can start after ~2 small transfers.
            HDL = DL // 2

            def src_ap(d_ap, row_len, n_c, col0, ncols, c0=0, row0=0):
                return bass.AP(
                    tensor=d_ap.tensor,
                    offset=(row0 + c0 * 128) * row_len + col0,
                    ap=[[row_len, 128], [row_len * 128, n_c], [1, ncols]],
                )

            def load(eng, dst, d_ap, row_len, n_c, col0, ncols, c0=0):
                eng.dma_start(dst, src_ap(d_ap, row_len, n_c, col0, ncols, c0))

            # all loads stay granular per c-tile: progressive sem arrival
            # keeps the PE p-state ramp behavior benign. x blocks 2-3 go on
            # the gpsimd (SWDGE) queue to unclog the shared HWDGE stage.
            for c0 in range(0, NCT, 2):
                load(nc.sync, xT_sb[:, c0 : c0 + 2, 0:QB], xT_d, T, 2, 0, QB, c0)
            # weight tiles pair-merged: halves the serialized DGE pipe cost
            # while keeping sem arrival granular enough for the consumers
            for c0 in range(0, NCT, 2):
                cs = slice(c0, c0 + 2)
                load(nc.scalar, wk_sb[:, cs, 0:HDL], wk_d, DL, 2, 0, HDL, c0)
                load(nc.gpsimd, wq_sb[:, cs, 0:HDL], wq_d, DL, 2, 0, HDL, c0)
                load(nc.gpsimd, wv_sb[:, cs, :], wv_d, DL, 2, 0, DL, c0)
            for tb in range(1, NQB):
                ts_ = slice(tb * QB, (tb + 1) * QB)
                for c in range(NCT):
                    nc.sync.dma_start(
                        xT_sb[:, c, ts_], xT_d[c * 128 : (c + 1) * 128, ts_]
                    )
            for c in range(NCT):
                cs = slice(c * 128, (c + 1) * 128)
                nc.scalar.dma_start(wk_sb[:, c, HDL:DL], wk_d[cs, HDL:DL])
                nc.gpsimd.dma_start(wq_sb[:, c, HDL:DL], wq_d[cs, HDL:DL])
            for r in range(DL // 128):
                nc.scalar.dma_start(wo_sb[:, r, :], wo_d[r * 128 : (r + 1) * 128, :])

            def proj_qk_block(w_sb, out_sb, j, tb):
                # (dl, t) projection for head pair j, one 512-col t block
                acc = ps_pool.tile([128, QB], f32, tag="ps")
                for c in range(NCT):
                    nc.tensor.matmul(
                        acc[:],
                        lhsT=w_sb[:, c, j * 128 : (j + 1) * 128],
                        rhs=xT_sb[:, c, tb * QB : (tb + 1) * QB],
                        start=(c == 0),
                        stop=(c == NCT - 1),
                    )
                    yield
                nc.vector.tensor_copy(out_sb[:, j, tb * QB : (tb + 1) * QB], acc[:])

            def proj_v_block(tt):
                # V natural: (t, dl) for one 128-row t tile, all heads
                acc = ps_pool.tile([128, DL], f32, tag="ps")
                for c in range(NCT):
                    nc.tensor.matmul(
                        acc[:],
                        lhsT=xT_sb[:, c, tt * 128 : (tt + 1) * 128],
                        rhs=wv_sb[:, c, :],
                        start=(c == 0),
                        stop=(c == NCT - 1),
                    )
                    yield
                nc.vector.tensor_copy(
                    V_sb[:, tt, :, 0:D],
                    acc.rearrange("p (h d) -> p h d", h=HL),
                )

            def proj_y_block(qb, ob):
                q0 = qb * QB
                acc = ps_pool.tile([128, QB], f32, tag="ps")
                for r in range(DL // 128):
                    nc.tensor.matmul(
                        acc[:],
                        lhsT=wo_sb[:, r, ob * 128 : (ob + 1) * 128],
                        rhs=AT_sb[:, r, q0 : q0 + QB],
                        start=(r == 0),
                        stop=(r == DL // 128 - 1),
                    )
                    yield
                yst = ystage.tile([128, QB], bf16, tag="yst")
                nc.vector.tensor_copy(yst[:], acc[:])
                nc.sync.dma_start(
                    yT_d[ob * 128 : (ob + 1) * 128, q0 : q0 + QB], yst[:]
                )

            # ---- filler machinery: a queue of (name, generator) projection
            # blocks streamed into the attention k-loop as PE gap filler ----
            filler: dict = {"items": [], "idx": 0, "done": set()}

            def filler_add(name, gen):
                filler["items"].append((name, gen))

            def filler_pull(n):
                pulled = 0
                while pulled < n and filler["idx"] < len(filler["items"]):
                    name, gen = filler["items"][filler["idx"]]
                    try:
                        next(gen)
                        pulled += 1
                    except StopIteration:
                        filler["done"].add(name)
                        filler["idx"] += 1

            def filler_flush_until(names):
                while not all(n in filler["done"] for n in names):
                    if filler["idx"] >= len(filler["items"]):
                        missing = [n for n in names if n not in filler["done"]]
                        raise RuntimeError(f"filler queue exhausted: {missing}")
                    filler_pull(1)

            # PSUM banks allow only ONE open accumulation group at a time,
            # so the flipped-AV accumulations are emitted as *consecutive*
            # instruction groups (phase B), interleaved on the PE with the
            # NEXT block's score/exp tiles (phase A) and projection fillers.
            # A single kernel-lifetime aux bank hosts the rotating transpose
            # outputs (single-shot matmuls only, never an open group).
            aux = aux_pool.tile([128, 384], f32, tag="aux", name="aux")
            aux_state = {"n": 0}
            # transposes of finished, normalized q-subtiles are deferred so
            # the PE never waits on the DVE norm mul
            pending_tp: list = []
            pending_tp_new: list = []

            def flush_tp(all_=False):
                while pending_tp:
                    pending_tp.pop(0)()
                pending_tp.extend(pending_tp_new)
                del pending_tp_new[:]
                if all_:
                    while pending_tp:
                        pending_tp.pop(0)()

            def norm_qt(j, qb, oq, s):
                # 1/l per query (l is col 64 of each accumulator slot), then
                # normalize into a bf16 staging tile; both heads at once.
                c0 = (2 * s) % 4
                rl = small.tile([128, 2, 1], f32, tag="rl", name="rl")
                rl_flat = bass.AP(
                    tensor=rl.tensor, offset=rl.offset, ap=[rl.ap[0], [1, 2]]
                )
                nc.vector.reciprocal(rl_flat, oq[:, c0 : c0 + 2, 64:65])
                aq = aq_pool.tile([128, 2, D], bf16, tag="aq", name="aq")
                rl_b = bass.AP(
                    tensor=rl.tensor,
                    offset=rl.offset,
                    ap=[rl.ap[0], rl.ap[1], [0, D]],
                )
                with nc.allow_low_precision(reason="bf16 attention out"):
                    nc.vector.tensor_mul(aq[:], oq[:, c0 : c0 + 2, 0:D], rl_b)

                def tp(aq=aq, s=s, j=j, q0=qb * QB):
                    # transpose [q, (h d)] -> [(h d), q] via matmul with the
                    # identity as moving operand; the two heads land on
                    # partition halves 0-63 / 64-127 as AT expects
                    col = 128 * (aux_state["n"] % 3)
                    aux_state["n"] += 1
                    aq_flat = bass.AP(
                        tensor=aq.tensor,
                        offset=aq.offset,
                        ap=[aq.ap[0], [1, 128]],
                    )
                    nc.tensor.matmul(
                        aux[:, col : col + 128],
                        lhsT=aq_flat,
                        rhs=ident_sb[:],
                        start=True,
                        stop=True,
                    )
                    nc.vector.tensor_copy(
                        AT_sb[:, j, q0 + s * 128 : q0 + (s + 1) * 128],
                        aux[:, col : col + 128],
                    )

                pending_tp_new.append(tp)

            def av_gen(j, qb):
                # phase B: flipped AV. Per query subtile s and head, one
                # consecutive accumulation group over the causal key tiles:
                #   oq[q, 0:64] += pt[:, hi, qsub].T @ [V_h | 1]
                # The moving dim is 65 while the contraction is the full 128
                # keys, so this runs at ~2x the PE rate of the [d, q]-layout
                # formulation. Column 64 collects l = sum(p).
                oq = oq_pool.tile([128, 4, D + 1], f32, tag="oq", name="oq")
                for s in range(4):
                    for hi in (0, 1):
                        slot = (2 * s + hi) % 4
                        kn = 4 * qb + s + 1
                        for k in range(kn):
                            nc.tensor.matmul(
                                oq[:, slot, :],
                                lhsT=pt_of[j, qb, k][:, hi, s * 128 : (s + 1) * 128],
                                rhs=V_sb[:, k, 2 * j + hi, :],
                                start=(k == 0),
                                stop=(k == kn - 1),
                            )
                            # the norm must be emitted before the final
                            # yield: the puller stops exactly at the yield
                            # count, so anything after the last yield would
                            # never run for non-final blocks
                            if hi == 1 and k == kn - 1:
                                norm_qt(j, qb, oq, s)
                            yield

            pt_of: dict = {}

            def pull_av(av, n):
                pulled = 0
                while pulled < n and av["gen"] is not None:
                    try:
                        next(av["gen"])
                        av["rem"] -= 1
                        pulled += 1
                    except StopIteration:
                        av["gen"] = None
                        if av["post"] is not None:
                            av["post"]()

            def attention(j, qb, prev_av, pull_n=2):
                q0 = qb * QB
                kb = (qb + 1) * (QB // 128)  # causal reach in 128-key tiles

                def emit_st(k):
                    k0 = k * 128
                    # diagonal tiles: only queries >= k0 are unmasked
                    w0 = max(0, k0 - q0)  # first valid query column
                    st = ps_pool.tile([128, 2, QB], f32, tag="st")
                    for hi, base in ((0, 0), (1, 64)):
                        nc.tensor.matmul(
                            st[:, hi, w0:QB],
                            lhsT=KT_sb[base : base + 64, j, k0 : k0 + 128],
                            rhs=QT_sb[base : base + 64, j, q0 + w0 : q0 + QB],
                            start=True,
                            stop=True,
                        )
                    pt = pt_pool.tile([128, 2, QB], bf16, tag="pt")
                    # P = exp(S / sqrt(D)); scores are O(1) so skipping the
                    # max-subtraction is safe in fp32/bf16 range.
                    nc.scalar.activation(
                        pt[:, :, w0:QB], st[:, :, w0:QB], EXP, scale=0.125
                    )
                    if k0 >= q0:
                        # tile crosses the causal diagonal: zero key > query
                        # (DVE multiply by the precomputed relative mask,
                        # broadcast over the two heads via a 0-stride dim)
                        m_ap = bass.AP(
                            tensor=mask_sb.tensor,
                            offset=mask_sb.offset,
                            ap=[mask_sb.ap[0], [0, 2], [1, QB - w0]],
                        )
                        nc.vector.tensor_mul(pt[:, :, w0:QB], pt[:, :, w0:QB], m_ap)
                    pt_of[j, qb, k] = pt

                # phase A of this block, with the previous block's phase B
                # spread evenly across the score slots
                for k in range(kb):
                    emit_st(k)
                    if prev_av["gen"] is not None:
                        need = -(-prev_av["rem"] // (kb - k))  # ceil
                        pull_av(prev_av, need)
                    filler_pull(pull_n)
                    flush_tp()
                # force-finalize: drain any remainder and take the extra
                # next() that raises StopIteration so post() always fires
                pull_av(prev_av, 1_000_000_000)

                post = None
                if j == NJ - 1:

                    def post(qb=qb):
                        # transposes must be emitted before the y fillers
                        # that read their AT output exist in the queue
                        flush_tp(all_=True)
                        for ob in range(C // 128):
                            filler_add(f"y{qb}.{ob}", proj_y_block(qb, ob))

                return {"gen": av_gen(j, qb), "rem": 32 * qb + 20, "post": post}

            def run(gen):
                for _ in gen:
                    pass

            # Build the filler queue: everything except the j=0/qb=0
            # prerequisites, in rough just-in-time order.
            for qb in range(1, NQB):
                filler_add(f"kq0.{qb}k", proj_qk_block(wk_sb, KT_sb, 0, qb))
                filler_add(f"kq0.{qb}q", proj_qk_block(wq_sb, QT_sb, 0, qb))
                for tt in range(4 * qb, 4 * qb + 4):
                    filler_add(f"v{tt}", proj_v_block(tt))
            for j in range(1, NJ):
                for qb in range(NQB):
                    filler_add(f"kq{j}.{qb}k", proj_qk_block(wk_sb, KT_sb, j, qb))
                    filler_add(f"kq{j}.{qb}q", proj_qk_block(wq_sb, QT_sb, j, qb))
            # y blocks are appended only after the attention that writes
            # their AT_sb input has been emitted (program-order correctness)

            def need_attention(j, qb):
                if j == 0:
                    if qb == 0:
                        return []
                    names = [f"kq0.{t}k" for t in range(1, qb + 1)]
                    names += [f"kq0.{qb}q"]
                    names += [f"v{t}" for t in range(4, 4 * qb + 4)]
                    return names
                names = [f"kq{j}.{t}k" for t in range(qb + 1)]
                names += [f"kq{j}.{qb}q"]
                return names

            # j=0/qb=0 prerequisites emitted directly
            run(proj_qk_block(wk_sb, KT_sb, 0, 0))
            run(proj_qk_block(wq_sb, QT_sb, 0, 0))
            for tt in range(4):
                run(proj_v_block(tt))

            av = {"gen": None, "rem": 0, "post": None}
            for j in range(NJ):
                for qb in range(NQB):
                    filler_flush_until(need_attention(j, qb))
                    # hold filler reserve through (2,3) so attention(3,0)
                    # still has PE cover before its y fillers exist
                    av = attention(
                        j, qb, av, pull_n=1 if (j, qb) == (2, 3) else 2
                    )
            # drain the last block's AV, transposes, and the tail y projs
            pull_av(av, 1_000_000_000)
            flush_tp(all_=True)
            filler_pull(1_000_000_000)

    nc.compile()
    return nc


def _get_nc():
    if "nc" not in _CACHE:
        _CACHE["nc"] = _build_nc()
    return _CACHE["nc"]


def _run(in_maps, trace=False):
    from concourse.bass_utils import run_bass_kernel_spmd

    nc = _get_nc()
    return run_bass_kernel_spmd(nc, in_maps, list(range(N_CORES)), trace=trace)


def _make_in_maps(x, W_Q, W_K, W_V, W_out):
    x = np.asarray(x, dtype=np.float32)
    W_Q = np.asarray(W_Q, dtype=np.float32)
    W_K = np.asarray(W_K, dtype=np.float32)
    W_V = np.asarray(W_V, dtype=np.float32)
    W_out = np.asarray(W_out, dtype=np.float32)

    in_maps = []
    for core in range(N_CORES):
        b, hh = core // 2, core % 2
        sl = slice(hh * DL, (hh + 1) * DL)
        in_maps.append(
            {
                "xt": np.ascontiguousarray(x[b].T).astype(BF16),
                "wqt": np.ascontiguousarray(W_Q[sl, :].T).astype(BF16),
                "wkt": np.ascontiguousarray(W_K[sl, :].T).astype(BF16),
                "wvt": np.ascontiguousarray(W_V[sl, :].T).astype(BF16),
                "wot": np.ascontiguousarray(W_out[:, sl].T).astype(BF16),
            }
        )
    return in_maps


def _assemble(results):
    y = np.empty((B, T, C), dtype=np.float32)
    for b in range(B):
        yT = results[2 * b]["yt"].astype(np.float32) + results[
            2 * b + 1
        ]["yt"].astype(np.float32)
        y[b] = yT.T
    return y


def kernel(x, W_Q, W_K, W_V, W_out):
    res = _run(_make_in_maps(x, W_Q, W_K, W_V, W_out), trace=False)
    return _assemble(res.results)
